# revision 33
# baseline (speedup 1.0000x reference)
"""MDCT kernel for Trainium2 (8 NeuronCores, batch-parallel), folded DCT-IV form.

Math: frame f (hop 1024, frame len 2048, center-padded) reduces via the
standard MDCT fold to a 1024-point DCT-IV:
    out[f, k] = sum_j S[j, k] * u_f[j],   S[j, k] = sqrt(2/N) cos(pi/N (j+.5)(k+.5))
    u_f = [P[f], Q[f-1]]                  (N = 1024, P[1024] = Q[-1] = 0)
with per-row folds of X2 = x.reshape(1024, 1024):
    y_r = w[1024:] * X2[r],  P[r, j] = -(y_r[511-j] + y_r[512+j])
    z_r = w[:1024] * X2[r],  Q[r, j] =   z_r[j]     - z_r[1023-j]
S is window-independent, so it ships as a precomputed bf16 constant and the
windowing runs on-chip, halving tensor-engine FLOPs vs the fused 2048-wide
basis.  x and the window ship as bf16 (the fold is bf16 on-chip regardless),
halving input DMA.

Schedule: depth-4 software pipeline over 128-row x tiles —
x-DMA(j+3) | fold(j+2) | transpose+copy(j+1) | matmul(j).  Folds run in bf16
on the vector engine (~2.4us/tile, under the ~3.9us PE loop of 16 matmuls +
8 transposes); PE transposes land in two shared PSUM tiles drained by one 3D
scalar-engine copy each; PSUM->SBUF output copies also on scalar.  Each DMA
dispatch queue stripes over all 16 DMA engines but sustains only ~150-200
GB/s, so traffic is spread across all three queues (sync: x + pa-half
outputs; scalar: S hi-half; gpsimd: window + S lo-half + pb-half outputs)
and output tiles go out as two half-tile DMAs as each PSUM half drains.
The lone frame-1024 matmuls run last so the kernel tail is a 4 KB write,
not a 512 KB one.
"""

import numpy as np
import ml_dtypes

import concourse.bass as bass
import concourse.bacc as bacc
import concourse.mybir as mybir
import concourse.tile as tile
from concourse import masks
from concourse.bass_utils import run_bass_kernel_spmd

B = 8
T = 1 << 20
R = 1024          # rows of X2 per channel (T // hop)
CN = 1024         # row width (hop)
NF = 1025         # output frames
NK = 1024         # output bins
H = 512           # half row
F32 = mybir.dt.float32
BF16 = mybir.dt.bfloat16

_NC_CACHE = None


def build_nc() -> bass.Bass:
    nc = bacc.Bacc("TRN2", target_bir_lowering=False, debug=False)
    x = nc.dram_tensor("x", [R, CN], BF16, kind="ExternalInput").ap()
    wb = nc.dram_tensor("wb", [2 * CN], BF16, kind="ExternalInput").ap()
    s = nc.dram_tensor("s", [CN, NK], BF16, kind="ExternalInput").ap()
    out = nc.dram_tensor("out", [NF, NK], F32, kind="ExternalOutput").ap()

    with tile.TileContext(nc) as tc:
        with (
            tc.tile_pool(name="persist", bufs=1) as persist,
            tc.tile_pool(name="xin", bufs=3) as xin,
            tc.tile_pool(name="pq", bufs=3) as pq,
            tc.tile_pool(name="outp", bufs=3) as outp,
            tc.tile_pool(name="tps", bufs=2, space="PSUM") as tps,
            tc.tile_pool(name="mmps", bufs=4, space="PSUM") as mmps,
        ):
            st = persist.tile([128, 8, NK], BF16)
            s_r = s.rearrange("(jc p) k -> p jc k", p=128)
            wtb = persist.tile([128, 2 * CN], BF16)

            xts = {}

            def load_x(i: int, eng=nc.sync):
                xts[i] = xin.tile([128, CN], BF16, tag="xt", name=f"xt{i}")
                eng.dma_start(xts[i][:], x[i * 128:(i + 1) * 128, :])

            # Head-critical DMAs.  Queues fair-share their packet streams,
            # so the first fold's inputs (window + x0) each get a queue to
            # themselves: the window leads the scalar queue, x0 splits its
            # rows across sync and gpsimd; S halves and x1/x2 follow in
            # first-use order.
            nc.scalar.dma_start(wtb[:], wb[None, :].partition_broadcast(128))
            xts[0] = xin.tile([128, CN], BF16, tag="xt", name="xt0")
            nc.sync.dma_start(xts[0][0:64, :], x[0:64, :])
            nc.gpsimd.dma_start(xts[0][64:128, :], x[64:128, :])
            nc.scalar.dma_start(st[:, 4:6, :], s_r[:, 4:6, :])
            nc.gpsimd.dma_start(st[:, 0:2, :], s_r[:, 0:2, :])
            load_x(1)
            nc.scalar.dma_start(st[:, 6:8, :], s_r[:, 6:8, :])
            nc.gpsimd.dma_start(st[:, 2:4, :], s_r[:, 2:4, :])
            load_x(2)

            ident = persist.tile([128, 128], BF16)
            masks.make_identity(nc, ident[:])

            # PTs[p, cc, r]: P[r, 128*cc + p]   (frame f reads col f)
            # QTs[p, cc, rr]: Q[rr-1, 128*cc+p] (frame f reads col f; col 0 = 0)
            pts = persist.tile([128, 4, R], BF16)
            qts = persist.tile([128, 4, R + 1], BF16)
            nc.vector.memset(qts[:, :, 0:1], 0.0)

            pqs = {}

            def fold_tile(i: int):
                """bf16 fold of x rows [128*i, 128*(i+1)) on the DVE."""
                xbt = xts.pop(i)
                zt = pq.tile([128, CN], BF16, tag="z")
                yt = pq.tile([128, CN], BF16, tag="y")
                nc.vector.tensor_tensor(zt[:], xbt[:], wtb[:, 0:CN],
                                        mybir.AluOpType.mult)
                nc.vector.tensor_tensor(yt[:], xbt[:], wtb[:, CN:2 * CN],
                                        mybir.AluOpType.mult)
                pt = pq.tile([128, H], BF16, tag="p")
                qt = pq.tile([128, H], BF16, tag="q")
                nc.vector.tensor_tensor(
                    qt[:], zt[:, 0:H], zt[:, CN - 1:H - 1:-1],
                    mybir.AluOpType.subtract,
                )
                # P = -(y_rev_lo + y_hi) = (y_rev_lo * -1) - y_hi
                nc.vector.scalar_tensor_tensor(
                    pt[:], yt[:, H - 1::-1], -1.0, yt[:, H:CN],
                    mybir.AluOpType.mult, mybir.AluOpType.subtract,
                )
                pqs[i] = (pt, qt)

            def tp_tile(i: int):
                """PE-transpose P/Q chunks of tile i, drain into PTs/QTs."""
                pt, qt = pqs.pop(i)
                ptp = tps.tile([128, H], BF16, tag="ptp")
                qtp = tps.tile([128, H], BF16, tag="qtp")
                for cc in range(4):
                    nc.tensor.transpose(
                        ptp[:, cc * 128:(cc + 1) * 128],
                        pt[:, cc * 128:(cc + 1) * 128], ident[:],
                    )
                    nc.tensor.transpose(
                        qtp[:, cc * 128:(cc + 1) * 128],
                        qt[:, cc * 128:(cc + 1) * 128], ident[:],
                    )
                nc.vector.tensor_copy(
                    pts[:, :, i * 128:(i + 1) * 128],
                    ptp[:].rearrange("p (c r) -> p c r", c=4),
                )
                nc.scalar.copy(
                    qts[:, :, 1 + i * 128:1 + (i + 1) * 128],
                    qtp[:].rearrange("p (c r) -> p c r", c=4),
                )

            def mm_tile(j: int):
                """Matmuls for frames [128j, 128j+128), grouped by k-half;
                each half's copy + DMA overlaps the other half's matmuls."""
                f0 = j * 128
                ot = outp.tile([128, NK], F32)
                for half, k0 in ((0, 0), (1, H)):
                    ph = mmps.tile([128, H], F32, tag="mm")
                    for ci in range(8):
                        if ci < 4:  # Q term first: S rows 512.. arrive first
                            lhsT = qts[:, ci, f0:f0 + 128]
                            rhs_c = 4 + ci
                        else:
                            lhsT = pts[:, ci - 4, f0:f0 + 128]
                            rhs_c = ci - 4
                        nc.tensor.matmul(
                            ph[:], lhsT, st[:, rhs_c, k0:k0 + H],
                            start=(ci == 0), stop=(ci == 7),
                        )
                    nc.scalar.copy(ot[:, k0:k0 + H], ph[:])
                    eng = nc.gpsimd if half == 0 else nc.scalar
                    if j == 7:  # split the tail writes across both queues
                        eng.dma_start(out[f0:f0 + 64, k0:k0 + H],
                                      ot[0:64, k0:k0 + H])
                        other = nc.scalar if half == 0 else nc.gpsimd
                        other.dma_start(out[f0 + 64:f0 + 128, k0:k0 + H],
                                        ot[64:128, k0:k0 + H])
                    else:
                        eng.dma_start(out[f0:f0 + 128, k0:k0 + H],
                                      ot[:, k0:k0 + H])

            def mm_last():
                """Frame 1024: P[1024] = 0, only the Q term contributes."""
                ot = outp.tile([1, NK], F32, tag="ot_last")
                for half, k0 in ((0, 0), (1, H)):
                    ph = mmps.tile([1, H], F32, tag="mm")
                    for cc in range(4):
                        nc.tensor.matmul(ph[:], qts[:, cc, R:R + 1],
                                         st[:, 4 + cc, k0:k0 + H],
                                         start=(cc == 0), stop=(cc == 3))
                    nc.scalar.copy(ot[:, k0:k0 + H], ph[:])
                nc.gpsimd.dma_start(out[R:R + 1, :], ot[:])

            # Prologue: fill the pipeline.
            fold_tile(0)
            tp_tile(0)
            fold_tile(1)

            for j in range(8):
                if j < 5:
                    load_x(j + 3)
                if j < 6:
                    fold_tile(j + 2)
                if j == 7:
                    mm_last()
                mm_tile(j)
                if j < 7:
                    tp_tile(j + 1)

    return nc


def make_s() -> np.ndarray:
    j = np.arange(CN, dtype=np.float64)[:, None]
    k = np.arange(NK, dtype=np.float64)[None, :]
    s = np.sqrt(2.0 / NK) * np.cos(np.pi / NK * (j + 0.5) * (k + 0.5))
    return s.astype(ml_dtypes.bfloat16)


_S = make_s()


def _get_nc() -> bass.Bass:
    global _NC_CACHE
    if _NC_CACHE is None:
        _NC_CACHE = build_nc()
        _NC_CACHE.compile()
    return _NC_CACHE


def run_spmd(x: np.ndarray, window: np.ndarray, **kwargs):
    """Shard, run on 8 cores, return (stacked output, BassKernelResults)."""
    wb = np.ascontiguousarray(window).astype(ml_dtypes.bfloat16)
    in_maps = [
        {"x": np.ascontiguousarray(x[b].reshape(R, CN)).astype(ml_dtypes.bfloat16),
         "wb": wb, "s": _S}
        for b in range(B)
    ]
    res = run_bass_kernel_spmd(nc=_get_nc(), in_maps=in_maps,
                               core_ids=list(range(B)), **kwargs)
    out = np.stack([res.results[b]["out"] for b in range(B)], axis=0)
    return out, res


def kernel(x: np.ndarray, window: np.ndarray) -> np.ndarray:
    out, _ = run_spmd(np.asarray(x), np.asarray(window))
    return out


# revision 36
# speedup vs baseline: 1.0424x; 1.0424x over previous
"""MDCT kernel for Trainium2 (8 NeuronCores, batch-parallel), folded DCT-IV form.

Math: frame f (hop 1024, frame len 2048, center-padded) reduces via the
standard MDCT fold to a 1024-point DCT-IV:
    out[f, k] = sum_j S[j, k] * u_f[j],   S[j, k] = sqrt(2/N) cos(pi/N (j+.5)(k+.5))
    u_f = [P[f], Q[f-1]]                  (N = 1024, P[1024] = Q[-1] = 0)
with per-row folds of X2 = x.reshape(1024, 1024):
    y_r = w[1024:] * X2[r],  P[r, j] = -(y_r[511-j] + y_r[512+j])
    z_r = w[:1024] * X2[r],  Q[r, j] =   z_r[j]     - z_r[1023-j]
S is window-independent, so it ships as a precomputed bf16 constant and the
windowing runs on-chip, halving tensor-engine FLOPs vs the fused 2048-wide
basis.  x and the window ship as bf16 (the fold is bf16 on-chip regardless),
halving input DMA.

Schedule: depth-4 software pipeline over 128-row x tiles —
x-DMA(j+3) | fold(j+2) | transpose+copy(j+1) | matmul(j).  Folds run in bf16
on the vector engine (~2.4us/tile, under the ~3.9us PE loop of 16 matmuls +
8 transposes); PE transposes land in two shared PSUM tiles drained by one 3D
scalar-engine copy each; PSUM->SBUF output copies also on scalar.  Each DMA
dispatch queue stripes over all 16 DMA engines but sustains only ~150-200
GB/s, so traffic is spread across all three queues (sync: x + pa-half
outputs; scalar: S hi-half; gpsimd: window + S lo-half + pb-half outputs)
and output tiles go out as two half-tile DMAs as each PSUM half drains.
The lone frame-1024 matmuls run last so the kernel tail is a 4 KB write,
not a 512 KB one.
"""

import numpy as np
import ml_dtypes

import concourse.bass as bass
import concourse.bacc as bacc
import concourse.mybir as mybir
import concourse.tile as tile
from concourse import masks
from concourse.bass_utils import run_bass_kernel_spmd

B = 8
T = 1 << 20
R = 1024          # rows of X2 per channel (T // hop)
CN = 1024         # row width (hop)
NF = 1025         # output frames
NK = 1024         # output bins
H = 512           # half row
F32 = mybir.dt.float32
BF16 = mybir.dt.bfloat16

_NC_CACHE = None


def build_nc() -> bass.Bass:
    nc = bacc.Bacc("TRN2", target_bir_lowering=False, debug=False)
    x = nc.dram_tensor("x", [R, CN], BF16, kind="ExternalInput").ap()
    wb = nc.dram_tensor("wb", [2 * CN], BF16, kind="ExternalInput").ap()
    s = nc.dram_tensor("s", [CN, NK], BF16, kind="ExternalInput").ap()
    out = nc.dram_tensor("out", [NF, NK], F32, kind="ExternalOutput").ap()

    with tile.TileContext(nc) as tc:
        with (
            tc.tile_pool(name="persist", bufs=1) as persist,
            tc.tile_pool(name="xin", bufs=3) as xin,
            tc.tile_pool(name="pq", bufs=3) as pq,
            tc.tile_pool(name="outp", bufs=3) as outp,
            tc.tile_pool(name="tps", bufs=2, space="PSUM") as tps,
            tc.tile_pool(name="mmps", bufs=4, space="PSUM") as mmps,
        ):
            st = persist.tile([128, 8, NK], BF16)
            s_r = s.rearrange("(jc p) k -> p jc k", p=128)
            wtb = persist.tile([128, 2 * CN], BF16)

            xts = {}

            def load_x(i: int, eng=nc.sync):
                xts[i] = xin.tile([128, CN], BF16, tag="xt", name=f"xt{i}")
                eng.dma_start(xts[i][:], x[i * 128:(i + 1) * 128, :])

            # Head-critical DMAs: x0 rides the sync queue alone (queues
            # fair-share their packet streams, so a queue-mate would delay
            # x0's completion to its own); x1/x2 slot between the S halves
            # on the other two queues, ordered to match first-use times.
            # The window is 4 KB — it lands instantly on the scalar queue
            # and is partition-replicated by a K=1 ones-matmul on the
            # otherwise-idle PE instead of a slow 0.5 MB broadcast DMA.
            wsb = persist.tile([1, 2 * CN], BF16)
            load_x(0)
            nc.scalar.dma_start(wsb[:], wb[None, :])
            nc.scalar.dma_start(st[:, 4:6, :], s_r[:, 4:6, :])
            load_x(1, nc.scalar)
            nc.scalar.dma_start(st[:, 6:8, :], s_r[:, 6:8, :])
            nc.gpsimd.dma_start(st[:, 0:2, :], s_r[:, 0:2, :])
            nc.gpsimd.dma_start(st[:, 2:4, :], s_r[:, 2:4, :])
            load_x(2, nc.gpsimd)

            ones = persist.tile([1, 128], BF16)
            nc.vector.memset(ones[:], 1.0)
            for c4 in range(4):
                wps = mmps.tile([128, H], F32, tag="mm")
                nc.tensor.matmul(wps[:], ones[:], wsb[:, c4 * H:(c4 + 1) * H],
                                 start=True, stop=True)
                nc.vector.tensor_copy(wtb[:, c4 * H:(c4 + 1) * H], wps[:])

            ident = persist.tile([128, 128], BF16)
            masks.make_identity(nc, ident[:])

            # PTs[p, cc, r]: P[r, 128*cc + p]   (frame f reads col f)
            # QTs[p, cc, rr]: Q[rr-1, 128*cc+p] (frame f reads col f; col 0 = 0)
            pts = persist.tile([128, 4, R], BF16)
            qts = persist.tile([128, 4, R + 1], BF16)
            nc.vector.memset(qts[:, :, 0:1], 0.0)

            pqs = {}

            def fold_tile(i: int):
                """bf16 fold of x rows [128*i, 128*(i+1)) on the DVE."""
                xbt = xts.pop(i)
                zt = pq.tile([128, CN], BF16, tag="z")
                yt = pq.tile([128, CN], BF16, tag="y")
                nc.vector.tensor_tensor(zt[:], xbt[:], wtb[:, 0:CN],
                                        mybir.AluOpType.mult)
                nc.vector.tensor_tensor(yt[:], xbt[:], wtb[:, CN:2 * CN],
                                        mybir.AluOpType.mult)
                pt = pq.tile([128, H], BF16, tag="p")
                qt = pq.tile([128, H], BF16, tag="q")
                nc.vector.tensor_tensor(
                    qt[:], zt[:, 0:H], zt[:, CN - 1:H - 1:-1],
                    mybir.AluOpType.subtract,
                )
                # P = -(y_rev_lo + y_hi) = (y_rev_lo * -1) - y_hi
                nc.vector.scalar_tensor_tensor(
                    pt[:], yt[:, H - 1::-1], -1.0, yt[:, H:CN],
                    mybir.AluOpType.mult, mybir.AluOpType.subtract,
                )
                pqs[i] = (pt, qt)

            def tp_tile(i: int):
                """PE-transpose P/Q chunks of tile i, drain into PTs/QTs."""
                pt, qt = pqs.pop(i)
                ptp = tps.tile([128, H], BF16, tag="ptp")
                qtp = tps.tile([128, H], BF16, tag="qtp")
                for cc in range(4):
                    nc.tensor.transpose(
                        ptp[:, cc * 128:(cc + 1) * 128],
                        pt[:, cc * 128:(cc + 1) * 128], ident[:],
                    )
                    nc.tensor.transpose(
                        qtp[:, cc * 128:(cc + 1) * 128],
                        qt[:, cc * 128:(cc + 1) * 128], ident[:],
                    )
                nc.vector.tensor_copy(
                    pts[:, :, i * 128:(i + 1) * 128],
                    ptp[:].rearrange("p (c r) -> p c r", c=4),
                )
                nc.scalar.copy(
                    qts[:, :, 1 + i * 128:1 + (i + 1) * 128],
                    qtp[:].rearrange("p (c r) -> p c r", c=4),
                )

            def mm_tile(j: int):
                """Matmuls for frames [128j, 128j+128), grouped by k-half;
                each half's copy + DMA overlaps the other half's matmuls."""
                f0 = j * 128
                ot = outp.tile([128, NK], F32)
                for half, k0 in ((0, 0), (1, H)):
                    ph = mmps.tile([128, H], F32, tag="mm")
                    for ci in range(8):
                        if ci < 4:  # Q term first: S rows 512.. arrive first
                            lhsT = qts[:, ci, f0:f0 + 128]
                            rhs_c = 4 + ci
                        else:
                            lhsT = pts[:, ci - 4, f0:f0 + 128]
                            rhs_c = ci - 4
                        nc.tensor.matmul(
                            ph[:], lhsT, st[:, rhs_c, k0:k0 + H],
                            start=(ci == 0), stop=(ci == 7),
                        )
                    nc.scalar.copy(ot[:, k0:k0 + H], ph[:])
                    eng = nc.gpsimd if half == 0 else nc.scalar
                    eng.dma_start(out[f0:f0 + 128, k0:k0 + H],
                                  ot[:, k0:k0 + H])

            def mm_last():
                """Frame 1024: P[1024] = 0, only the Q term contributes."""
                ot = outp.tile([1, NK], F32, tag="ot_last")
                for half, k0 in ((0, 0), (1, H)):
                    ph = mmps.tile([1, H], F32, tag="mm")
                    for cc in range(4):
                        nc.tensor.matmul(ph[:], qts[:, cc, R:R + 1],
                                         st[:, 4 + cc, k0:k0 + H],
                                         start=(cc == 0), stop=(cc == 3))
                    nc.scalar.copy(ot[:, k0:k0 + H], ph[:])
                nc.gpsimd.dma_start(out[R:R + 1, :], ot[:])

            # Prologue: fill the pipeline.
            fold_tile(0)
            tp_tile(0)
            fold_tile(1)

            for j in range(8):
                if j < 5:
                    load_x(j + 3)
                if j < 6:
                    fold_tile(j + 2)
                if j == 7:
                    mm_last()
                mm_tile(j)
                if j < 7:
                    tp_tile(j + 1)

    return nc


def make_s() -> np.ndarray:
    j = np.arange(CN, dtype=np.float64)[:, None]
    k = np.arange(NK, dtype=np.float64)[None, :]
    s = np.sqrt(2.0 / NK) * np.cos(np.pi / NK * (j + 0.5) * (k + 0.5))
    return s.astype(ml_dtypes.bfloat16)


_S = make_s()


def _get_nc() -> bass.Bass:
    global _NC_CACHE
    if _NC_CACHE is None:
        _NC_CACHE = build_nc()
        _NC_CACHE.compile()
    return _NC_CACHE


def run_spmd(x: np.ndarray, window: np.ndarray, **kwargs):
    """Shard, run on 8 cores, return (stacked output, BassKernelResults)."""
    wb = np.ascontiguousarray(window).astype(ml_dtypes.bfloat16)
    in_maps = [
        {"x": np.ascontiguousarray(x[b].reshape(R, CN)).astype(ml_dtypes.bfloat16),
         "wb": wb, "s": _S}
        for b in range(B)
    ]
    res = run_bass_kernel_spmd(nc=_get_nc(), in_maps=in_maps,
                               core_ids=list(range(B)), **kwargs)
    out = np.stack([res.results[b]["out"] for b in range(B)], axis=0)
    return out, res


def kernel(x: np.ndarray, window: np.ndarray) -> np.ndarray:
    out, _ = run_spmd(np.asarray(x), np.asarray(window))
    return out


# revision 37
# speedup vs baseline: 1.0485x; 1.0058x over previous
"""MDCT kernel for Trainium2 (8 NeuronCores, batch-parallel), folded DCT-IV form.

Math: frame f (hop 1024, frame len 2048, center-padded) reduces via the
standard MDCT fold to a 1024-point DCT-IV:
    out[f, k] = sum_j S[j, k] * u_f[j],   S[j, k] = sqrt(2/N) cos(pi/N (j+.5)(k+.5))
    u_f = [P[f], Q[f-1]]                  (N = 1024, P[1024] = Q[-1] = 0)
with per-row folds of X2 = x.reshape(1024, 1024):
    y_r = w[1024:] * X2[r],  P[r, j] = -(y_r[511-j] + y_r[512+j])
    z_r = w[:1024] * X2[r],  Q[r, j] =   z_r[j]     - z_r[1023-j]
S is window-independent, so it ships as a precomputed bf16 constant and the
windowing runs on-chip, halving tensor-engine FLOPs vs the fused 2048-wide
basis.  x and the window ship as bf16 (the fold is bf16 on-chip regardless),
halving input DMA.

Schedule: depth-4 software pipeline over 128-row x tiles —
x-DMA(j+3) | fold(j+2) | transpose+copy(j+1) | matmul(j).  Folds run in bf16
on the vector engine (~2.4us/tile, under the ~3.9us PE loop of 16 matmuls +
8 transposes); PE transposes land in two shared PSUM tiles drained by one 3D
scalar-engine copy each; PSUM->SBUF output copies also on scalar.  Each DMA
dispatch queue stripes over all 16 DMA engines but sustains only ~150-200
GB/s, so traffic is spread across all three queues (sync: x + pa-half
outputs; scalar: S hi-half; gpsimd: window + S lo-half + pb-half outputs)
and output tiles go out as two half-tile DMAs as each PSUM half drains.
The lone frame-1024 matmuls run last so the kernel tail is a 4 KB write,
not a 512 KB one.
"""

import numpy as np
import ml_dtypes

import concourse.bass as bass
import concourse.bacc as bacc
import concourse.mybir as mybir
import concourse.tile as tile
from concourse import masks
from concourse.bass_utils import run_bass_kernel_spmd

B = 8
T = 1 << 20
R = 1024          # rows of X2 per channel (T // hop)
CN = 1024         # row width (hop)
NF = 1025         # output frames
NK = 1024         # output bins
H = 512           # half row
F32 = mybir.dt.float32
BF16 = mybir.dt.bfloat16

_NC_CACHE = None


def build_nc() -> bass.Bass:
    nc = bacc.Bacc("TRN2", target_bir_lowering=False, debug=False)
    x = nc.dram_tensor("x", [R, CN], BF16, kind="ExternalInput").ap()
    wb = nc.dram_tensor("wb", [2 * CN], BF16, kind="ExternalInput").ap()
    s = nc.dram_tensor("s", [CN, NK], BF16, kind="ExternalInput").ap()
    out = nc.dram_tensor("out", [NF, NK], F32, kind="ExternalOutput").ap()

    with tile.TileContext(nc) as tc:
        with (
            tc.tile_pool(name="persist", bufs=1) as persist,
            tc.tile_pool(name="xin", bufs=3) as xin,
            tc.tile_pool(name="pq", bufs=3) as pq,
            tc.tile_pool(name="outp", bufs=3) as outp,
            tc.tile_pool(name="tps", bufs=2, space="PSUM") as tps,
            tc.tile_pool(name="mmps", bufs=4, space="PSUM") as mmps,
        ):
            st = persist.tile([128, 8, NK], BF16)
            s_r = s.rearrange("(jc p) k -> p jc k", p=128)
            wtb = persist.tile([128, 2 * CN], BF16)

            xts = {}

            def load_x(i: int, eng=nc.sync):
                xts[i] = xin.tile([128, CN], BF16, tag="xt", name=f"xt{i}")
                eng.dma_start(xts[i][:], x[i * 128:(i + 1) * 128, :])

            # Head-critical DMAs: x0 rides the sync queue alone (queues
            # fair-share their packet streams, so a queue-mate would delay
            # x0's completion to its own); x1/x2 slot between the S halves
            # on the other two queues, ordered to match first-use times.
            load_x(0)
            nc.scalar.dma_start(st[:, 4:6, :], s_r[:, 4:6, :])
            nc.gpsimd.dma_start(wtb[:], wb[None, :].partition_broadcast(128))
            load_x(1, nc.scalar)
            nc.scalar.dma_start(st[:, 6:8, :], s_r[:, 6:8, :])
            nc.gpsimd.dma_start(st[:, 0:2, :], s_r[:, 0:2, :])
            nc.gpsimd.dma_start(st[:, 2:4, :], s_r[:, 2:4, :])
            load_x(2, nc.gpsimd)

            ident = persist.tile([128, 128], BF16)
            masks.make_identity(nc, ident[:])

            # PTs[p, cc, r]: P[r, 128*cc + p]   (frame f reads col f)
            # QTs[p, cc, rr]: Q[rr-1, 128*cc+p] (frame f reads col f; col 0 = 0)
            pts = persist.tile([128, 4, R], BF16)
            qts = persist.tile([128, 4, R + 1], BF16)
            nc.vector.memset(qts[:, :, 0:1], 0.0)

            pqs = {}

            def fold_tile(i: int):
                """bf16 fold of x rows [128*i, 128*(i+1)) on the DVE."""
                xbt = xts.pop(i)
                zt = pq.tile([128, CN], BF16, tag="z")
                yt = pq.tile([128, CN], BF16, tag="y")
                nc.vector.tensor_tensor(zt[:], xbt[:], wtb[:, 0:CN],
                                        mybir.AluOpType.mult)
                nc.vector.tensor_tensor(yt[:], xbt[:], wtb[:, CN:2 * CN],
                                        mybir.AluOpType.mult)
                pt = pq.tile([128, H], BF16, tag="p")
                qt = pq.tile([128, H], BF16, tag="q")
                nc.vector.tensor_tensor(
                    qt[:], zt[:, 0:H], zt[:, CN - 1:H - 1:-1],
                    mybir.AluOpType.subtract,
                )
                # P = -(y_rev_lo + y_hi) = (y_rev_lo * -1) - y_hi
                nc.vector.scalar_tensor_tensor(
                    pt[:], yt[:, H - 1::-1], -1.0, yt[:, H:CN],
                    mybir.AluOpType.mult, mybir.AluOpType.subtract,
                )
                pqs[i] = (pt, qt)

            def tp_tile(i: int):
                """PE-transpose P/Q chunks of tile i, drain into PTs/QTs."""
                pt, qt = pqs.pop(i)
                ptp = tps.tile([128, H], BF16, tag="ptp")
                qtp = tps.tile([128, H], BF16, tag="qtp")
                for cc in range(4):
                    nc.tensor.transpose(
                        ptp[:, cc * 128:(cc + 1) * 128],
                        pt[:, cc * 128:(cc + 1) * 128], ident[:],
                    )
                    nc.tensor.transpose(
                        qtp[:, cc * 128:(cc + 1) * 128],
                        qt[:, cc * 128:(cc + 1) * 128], ident[:],
                    )
                nc.vector.tensor_copy(
                    pts[:, :, i * 128:(i + 1) * 128],
                    ptp[:].rearrange("p (c r) -> p c r", c=4),
                )
                nc.scalar.copy(
                    qts[:, :, 1 + i * 128:1 + (i + 1) * 128],
                    qtp[:].rearrange("p (c r) -> p c r", c=4),
                )

            def mm_tile(j: int):
                """Matmuls for frames [128j, 128j+128), grouped by k-half;
                each half's copy + DMA overlaps the other half's matmuls."""
                f0 = j * 128
                ot = outp.tile([128, NK], F32)
                for half, k0 in ((0, 0), (1, H)):
                    ph = mmps.tile([128, H], F32, tag="mm")
                    for ci in range(8):
                        if ci < 4:  # Q term first: S rows 512.. arrive first
                            lhsT = qts[:, ci, f0:f0 + 128]
                            rhs_c = 4 + ci
                        else:
                            lhsT = pts[:, ci - 4, f0:f0 + 128]
                            rhs_c = ci - 4
                        nc.tensor.matmul(
                            ph[:], lhsT, st[:, rhs_c, k0:k0 + H],
                            start=(ci == 0), stop=(ci == 7),
                        )
                    nc.scalar.copy(ot[:, k0:k0 + H], ph[:])
                    eng = nc.gpsimd if half == 0 else nc.scalar
                    eng.dma_start(out[f0:f0 + 128, k0:k0 + H],
                                  ot[:, k0:k0 + H])

            def mm_last():
                """Frame 1024: P[1024] = 0, only the Q term contributes."""
                ot = outp.tile([1, NK], F32, tag="ot_last")
                for half, k0 in ((0, 0), (1, H)):
                    ph = mmps.tile([1, H], F32, tag="mm")
                    for cc in range(4):
                        nc.tensor.matmul(ph[:], qts[:, cc, R:R + 1],
                                         st[:, 4 + cc, k0:k0 + H],
                                         start=(cc == 0), stop=(cc == 3))
                    nc.scalar.copy(ot[:, k0:k0 + H], ph[:])
                nc.gpsimd.dma_start(out[R:R + 1, :], ot[:])

            # Prologue: fill the pipeline.
            fold_tile(0)
            tp_tile(0)
            fold_tile(1)

            for j in range(8):
                if j < 5:
                    load_x(j + 3)
                if j < 6:
                    fold_tile(j + 2)
                if j == 7:
                    mm_last()
                mm_tile(j)
                if j < 7:
                    tp_tile(j + 1)

    return nc


def make_s() -> np.ndarray:
    j = np.arange(CN, dtype=np.float64)[:, None]
    k = np.arange(NK, dtype=np.float64)[None, :]
    s = np.sqrt(2.0 / NK) * np.cos(np.pi / NK * (j + 0.5) * (k + 0.5))
    return s.astype(ml_dtypes.bfloat16)


_S = make_s()


def _get_nc() -> bass.Bass:
    global _NC_CACHE
    if _NC_CACHE is None:
        _NC_CACHE = build_nc()
        _NC_CACHE.compile()
    return _NC_CACHE


def run_spmd(x: np.ndarray, window: np.ndarray, **kwargs):
    """Shard, run on 8 cores, return (stacked output, BassKernelResults)."""
    wb = np.ascontiguousarray(window).astype(ml_dtypes.bfloat16)
    in_maps = [
        {"x": np.ascontiguousarray(x[b].reshape(R, CN)).astype(ml_dtypes.bfloat16),
         "wb": wb, "s": _S}
        for b in range(B)
    ]
    res = run_bass_kernel_spmd(nc=_get_nc(), in_maps=in_maps,
                               core_ids=list(range(B)), **kwargs)
    out = np.stack([res.results[b]["out"] for b in range(B)], axis=0)
    return out, res


def kernel(x: np.ndarray, window: np.ndarray) -> np.ndarray:
    out, _ = run_spmd(np.asarray(x), np.asarray(window))
    return out
